# revision 21
# baseline (speedup 1.0000x reference)
"""MHSA block (b=8, c=256, h=w=32, nh=8) on 8 Trainium2 cores.

Sharding: pure data parallel -- one batch element per NeuronCore, no
collectives.  Per-core algorithm (X = x[b] as (C=256, L=1024)):

  QK    = Wqk @ X  (+bqk on the PSUM-evacuation add)              (512, L)
  V^T   = X^T @ WvT                                               (L, 256)
  S^T_h = K_h^T Q_h   4 heads concurrently via 4x row tiling      (128,1024)/jc
  P^T   = heads {0,1}: exp(scale*S^T) on ScalarE (ACT)
          heads {2,3}: Schraudolph bit-trick exp on DVE --
            int16(rint(s*scale*128*log2e + 16219)) bitcast as bf16
            == exp(scale*s)*(1 +- 3%).  Softmax ratio cancels most of
            the error (validated end-to-end ~3e-3 rel).  This halves
            the ScalarE exp floor, the baseline bottleneck.
  O_h   = V_h^T.T @ P^T_h  4 heads via col tiling, PSUM accum     (128, 512)/quad
  l     = ones(128,32)^T @ P^T_h col-tiled: ALL 32 rows of each
          head band get the denominator (pre-broadcast for free)
  O_n   = O * reciprocal_approx_fast(l)   (2 DVE ops per quad)
  out   = PSUM accumulation: I@bf16(x) + I@rb + Wp @ O_n, where
          rb = bf16(x - bf16(x) + bproj + Wproj@bv) is a host-side
          packed correction (keeps the residual exact to ~1e-4 while
          letting PSUM do the adds); evacuate once (ACT Identity), DMA.

Quads ih-major: (tg,ih) = (0,0),(1,0),(0,1),(1,1) so each column-half's
projection runs mid-window.  PSUM budget: 3x S^T pair double-buffer = 6
banks + PV accum + denom accum = 8.  QK / V^T / proj borrow stps slots.
"""

import sys
import os

sys.path.insert(0, "/opt/trn_rl_repo")

from contextlib import ExitStack

import numpy as np

NH, DH, C, L = 8, 32, 256, 1024
B = 8
SCALE = DH ** -0.5
N_CORES = 8
XW_W = 4384

# Schraudolph constants: bits = rint(s * SCHA + SCHB) viewed as bf16
SCHA = float(SCALE * 128.0 * 1.4426950408889634)
SCHB = 16219.0   # 16256 - 37 (centers the mantissa-linear error)

_CACHE = {}


def _build_nc():
    import concourse.tile as tile
    from concourse import bacc, mybir

    f32 = mybir.dt.float32
    bf16 = mybir.dt.bfloat16
    i16 = mybir.dt.int16
    Exp = mybir.ActivationFunctionType.Exp
    Identity = mybir.ActivationFunctionType.Identity
    Mult = mybir.AluOpType.mult
    Add = mybir.AluOpType.add

    nc = bacc.Bacc("TRN2", target_bir_lowering=False, debug=False)

    xw_d = nc.dram_tensor("xw", [128, XW_W], bf16, kind="ExternalInput").ap()
    rb_d = nc.dram_tensor("rb", [128, 2048], bf16, kind="ExternalInput").ap()
    bqkc_d = nc.dram_tensor("bqkc", [128, 4], f32, kind="ExternalInput").ap()
    out_d = nc.dram_tensor("out", [C, L], f32, kind="ExternalOutput").ap()

    with tile.TileContext(nc) as tc, ExitStack() as ctx:
        persist = ctx.enter_context(tc.tile_pool(name="persist", bufs=1))
        ptpool = ctx.enter_context(tc.tile_pool(name="pt", bufs=8))
        onpool = ctx.enter_context(tc.tile_pool(name="on", bufs=2))
        smallp = ctx.enter_context(tc.tile_pool(name="small", bufs=2))
        evpool = ctx.enter_context(tc.tile_pool(name="ev", bufs=2))
        stps = ctx.enter_context(tc.tile_pool(name="stps", bufs=3, space="PSUM"))
        pvps = ctx.enter_context(tc.tile_pool(name="pvps", bufs=1, space="PSUM"))

        def stps_borrow(name, long_lived=False):
            # Borrow one stps buffer but advance the tag rotation by a full
            # period (3) so the sts double-buffer pairing phase stays fixed:
            # tile A of slot s must always land on the buffer freed by the
            # schraudolph read two slots back, never on last slot's exp.
            # Long-lived borrows (pj) go LAST so the soonest sts reuse of
            # their buffer is a full period away.
            if long_lived:
                stps.tile([128, L], f32, tag="st", name=f"{name}_d1")
                stps.tile([128, L], f32, tag="st", name=f"{name}_d2")
                return stps.tile([128, L], f32, tag="st", name=name)
            t = stps.tile([128, L], f32, tag="st", name=name)
            stps.tile([128, L], f32, tag="st", name=f"{name}_d1")
            stps.tile([128, L], f32, tag="st", name=f"{name}_d2")
            return t

        xw = persist.tile([128, XW_W], bf16, tag="xw", name="xw")
        rb_sb = persist.tile([128, 2048], bf16, tag="rb", name="rb_sb")
        bqkc_sb = persist.tile([128, 4], f32, tag="bqkc", name="bqkc")

        # ---- input DMA, multi-engine issue; critical pieces first ----
        nc.sync.dma_start(xw[:, 0:512], xw_d[:, 0:512])            # x0 a-half
        nc.scalar.dma_start(xw[:, 1024:1536], xw_d[:, 1024:1536])  # x1 a-half
        nc.gpsimd.dma_start(xw[:, 2048:2176], xw_d[:, 2048:2176])  # wqk kt0 mt0
        nc.sync.dma_start(xw[:, 2560:2688], xw_d[:, 2560:2688])    # wqk kt1 mt0
        nc.gpsimd.dma_start(xw[:, 2304:2432], xw_d[:, 2304:2432])  # wqk kt0 mt2
        nc.scalar.dma_start(xw[:, 2816:2944], xw_d[:, 2816:2944])  # wqk kt1 mt2
        nc.gpsimd.dma_start(bqkc_sb[:], bqkc_d[:])
        nc.gpsimd.dma_start(xw[:, 3072:3584], xw_d[:, 3072:3584])  # wv (vt s1)
        # x b-halves (needed by slot 3 for K0b, slot 4 for vt quad b)
        nc.sync.dma_start(xw[:, 512:1024], xw_d[:, 512:1024])
        nc.scalar.dma_start(xw[:, 1536:2048], xw_d[:, 1536:2048])
        # Q1/K1 wqk chunks (needed by slots 5-8)
        nc.sync.dma_start(xw[:, 2176:2304], xw_d[:, 2176:2304])    # wqk kt0 mt1
        nc.scalar.dma_start(xw[:, 2688:2816], xw_d[:, 2688:2816])  # wqk kt1 mt1
        nc.gpsimd.dma_start(xw[:, 2432:2560], xw_d[:, 2432:2560])  # wqk kt0 mt3
        nc.sync.dma_start(xw[:, 2944:3072], xw_d[:, 2944:3072])    # wqk kt1 mt3
        # ident + ones + zl (ones/zl needed by pv at slot 2)
        nc.gpsimd.dma_start(xw[:, 4096:XW_W], xw_d[:, 4096:XW_W])
        # wp; rb (needed by proj ~slot 19)
        nc.scalar.dma_start(xw[:, 3584:4096], xw_d[:, 3584:4096])
        nc.sync.dma_start(rb_sb[:, 0:1024], rb_d[:, 0:1024])
        nc.gpsimd.dma_start(rb_sb[:, 1024:2048], rb_d[:, 1024:2048])

        x_sb = [xw[:, 0:1024], xw[:, 1024:2048]]
        wqk_sb_w = [xw[:, 2048:2560], xw[:, 2560:3072]]
        wv_sb = [xw[:, 3072:3328], xw[:, 3328:3584]]
        wp_sb = [xw[:, 3584:3840], xw[:, 3840:4096]]
        id_sb = xw[:, 4096:4224]    # 128x128 identity (proj residual preload)
        ones_sb = xw[:, 4224:4256]  # 128x32 all-ones (denominator lhsT)
        zl_sb = xw[:, 4256:4384]    # zero lhsT for PSUM pre-clear matmuls

        # warm the ACT exp table while the DMAs run (memsets on DVE: idle)
        warm = persist.tile([1, 8], f32, tag="warm", name="warm")
        nc.vector.memset(warm[:], 0.0)
        nc.scalar.activation(warm[:], warm[:], Exp)

        # ---- QK gemm:  QK(512, L) = WqkT.T @ X; bqk added on the evac. ----
        qk_sb = [persist.tile([128, L], bf16, tag=f"qk{mt}", name=f"qk{mt}")
                 for mt in range(4)]

        def qk_half(mt, half, use_act=False):
            pst = stps_borrow(f"qkps{mt}{half}")
            ps = pst[:, 0:512]
            for kt in range(2):
                nc.tensor.matmul(
                    ps,
                    lhsT=wqk_sb_w[kt][:, mt * 128:(mt + 1) * 128],
                    rhs=x_sb[kt][:, half * 512:(half + 1) * 512],
                    start=(kt == 0),
                    stop=(kt == 1),
                )
            o = qk_sb[mt][:, half * 512:(half + 1) * 512]
            if use_act:
                nc.scalar.activation(o, ps, Identity, bias=bqkc_sb[:, mt:mt + 1])
            else:
                nc.vector.tensor_scalar_add(o, ps, bqkc_sb[:, mt:mt + 1])

        def qk_pair(mt, use_act=False):
            # both column halves of one qk tile: 4 matmuls, ONE wide evac
            pst = stps_borrow(f"qkpp{mt}")
            for half in range(2):
                for kt in range(2):
                    nc.tensor.matmul(
                        pst[:, half * 512:(half + 1) * 512],
                        lhsT=wqk_sb_w[kt][:, mt * 128:(mt + 1) * 128],
                        rhs=x_sb[kt][:, half * 512:(half + 1) * 512],
                        start=(kt == 0),
                        stop=(kt == 1),
                    )
            if use_act:
                nc.scalar.activation(qk_sb[mt][:], pst[:], Identity,
                                     bias=bqkc_sb[:, mt:mt + 1])
            else:
                nc.vector.tensor_scalar_add(qk_sb[mt][:], pst[:],
                                            bqkc_sb[:, mt:mt + 1])

        # ---- V^T gemm: VT(L, 256) = X.T @ WvT (bias folded into rb) ----
        vt_sb = [None] * 8

        def vt_pair(jt):
            # two j-chunks share one PSUM tile and one ACT evacuation
            pst = stps_borrow(f"vtps{jt}")
            for s in range(2):
                for kt in range(2):
                    nc.tensor.matmul(
                        pst[:, s * 256:s * 256 + 256],
                        lhsT=x_sb[kt][:, (jt + s) * 128:(jt + s + 1) * 128],
                        rhs=wv_sb[kt],
                        start=(kt == 0),
                        stop=(kt == 1),
                    )
            vt = persist.tile([128, 512], bf16, tag=f"vt{jt}", name=f"vt{jt}")
            nc.scalar.activation(vt[:], pst[:, 0:512], Identity)
            vt_sb[jt] = vt[:, 0:256]
            vt_sb[jt + 1] = vt[:, 256:512]

        # PE warm-up so HAM grants full clock before the first QK matmul
        scratch = persist.tile([128, 512], bf16, tag="scratch", name="scratch")
        nc.vector.memset(scratch[:], 0.0)
        wpst = stps_borrow("warmps")
        wps = wpst[:, 0:512]
        for i in range(8):
            nc.tensor.matmul(wps, lhsT=scratch[:, 0:128], rhs=scratch[:],
                             start=(i == 0), stop=(i == 7))
        nc.vector.tensor_copy(scratch[:, 0:128], wps[:, 0:128])

        qk_half(2, 0, use_act=False)  # K first: its DVE evac unblocks S^T
        qk_half(0, 0, use_act=True)

        deferred = []
        on_holder = {}

        def make_chain_a_parts(tg, ih, qi, pv, dn):
            # dn rows 32m..32m+31 all hold l_h (all-ones lhsT), so the
            # normalization is reciprocal + tensor mul straight off PSUM.
            holder = {}

            def part(o, w, last):
                def fn():
                    if "rps" not in holder:
                        holder["rps"] = smallp.tile([128, 512], f32, tag="rps", name="rps")
                        holder["on"] = onpool.tile([128, 512], bf16, tag="on", name="on")
                    rps, on = holder["rps"], holder["on"]
                    nc.vector.reciprocal_approx_fast(rps[:, o:o + w], dn[:, o:o + w])
                    nc.vector.tensor_mul(on[:, o:o + w], pv[:, o:o + w], rps[:, o:o + w])
                    if last:
                        on_holder[(tg, ih)] = on

                return fn

            if qi == 3:
                return [part(0, 256, False), part(256, 256, True)]
            return [part(0, 512, True)]

        def make_chain_b_parts(ih, parts, fine_evac):
            cols0 = ih * 512
            holder = {}

            def part(o, w, first, last):
                def fn():
                    if first:
                        holder["on0"] = on_holder.pop((0, ih))
                        holder["on1"] = on_holder.pop((1, ih))
                        holder["pj"] = stps_borrow("pj", long_lived=True)
                        holder["ev"] = evpool.tile([128, L], f32, tag="ev", name="ev")
                    on0, on1 = holder["on0"], holder["on1"]
                    pj, ev = holder["pj"], holder["ev"]
                    for mt in range(2):
                        pjv = pj[:, mt * 512 + o:mt * 512 + o + w]
                        nc.tensor.matmul(
                            pjv, lhsT=id_sb,
                            rhs=x_sb[mt][:, cols0 + o:cols0 + o + w],
                            start=True, stop=False)
                        nc.tensor.matmul(
                            pjv, lhsT=id_sb,
                            rhs=rb_sb[:, mt * 1024 + cols0 + o:mt * 1024 + cols0 + o + w],
                            start=False, stop=False)
                        for tg, on in ((0, on0), (1, on1)):
                            nc.tensor.matmul(
                                pjv,
                                lhsT=wp_sb[tg][:, mt * 128:(mt + 1) * 128],
                                rhs=on[:, o:o + w],
                                start=False, stop=(tg == 1))
                        if fine_evac:
                            # evac + DMA right behind this mt's matmuls so
                            # ACT/DMA pipeline under the next mt's PE work
                            evv = ev[:, mt * 512 + o:mt * 512 + o + w]
                            nc.scalar.activation(evv, pjv, Identity)
                            eng = nc.sync if mt == 0 else nc.gpsimd
                            eng.dma_start(
                                out_d[mt * 128:(mt + 1) * 128,
                                      cols0 + o:cols0 + o + w], evv)
                    if not fine_evac and last:
                        nc.scalar.activation(ev[:], pj[:], Identity)
                        for mt in range(2):
                            eng = nc.sync if mt == 0 else nc.gpsimd
                            eng.dma_start(
                                out_d[mt * 128:(mt + 1) * 128, cols0:cols0 + 512],
                                ev[:, mt * 512:mt * 512 + 512])

                return fn

            return [part(o, w, o == parts[0][0], k == len(parts) - 1)
                    for k, (o, w) in enumerate(parts)]

        def make_quad(qi, tg, ih):
            cols = slice(ih * 512, (ih + 1) * 512)
            state = {}

            def st_fn(jc):
                for h in iter_hooks.pop((qi, jc), []):
                    h()
                qt = qk_sb[tg]
                kt_ = qk_sb[2 + tg]
                sts = [
                    stps.tile([128, L], f32, tag="st", name="stA"),
                    stps.tile([128, L], f32, tag="st", name="stB"),
                ]
                for m in range(4):
                    o = 32 * m
                    nc.tensor.matmul(
                        sts[m // 2][:, (m % 2) * 512:(m % 2) * 512 + 512],
                        lhsT=kt_[o:o + 32, jc * 128:(jc + 1) * 128],
                        rhs=qt[o:o + 32, cols],
                        start=True,
                        stop=True,
                        tile_position=(o, 0),
                    )
                # heads {0,1}: exact exp on ACT; heads {2,3}: Schraudolph on DVE
                ptA = ptpool.tile([128, L], bf16, tag="pta", name="pta")
                nc.scalar.activation(ptA[:], sts[0][:], Exp, scale=SCALE)
                ptB = ptpool.tile([128, L], i16, tag="ptb", name="ptb")
                nc.vector.tensor_scalar(ptB[:], sts[1][:], SCHA, SCHB, Mult, Add)
                state[jc] = (ptA, ptB)

            def pv_fn(jc):
                if jc == 0:
                    # Pre-zero both accumulators (concurrent col-tiled matmuls
                    # must not carry start=True).
                    pvdn = pvps.tile([128, 1024], f32, tag="pvdn", name="pvdn")
                    state["pv"] = pvdn[:, 0:512]
                    state["dn"] = pvdn[:, 512:1024]
                    for z0 in (0, 512):
                        nc.tensor.matmul(
                            pvdn[:, z0:z0 + 512],
                            lhsT=zl_sb,
                            rhs=x_sb[0][:, z0:z0 + 512],
                            start=True,
                            stop=True,
                        )
                ptA, ptB = state.pop(jc)
                pv, dn = state["pv"], state["dn"]

                def prhs(m):
                    c = (m % 2) * 512
                    if m // 2 == 0:
                        return ptA[:, c:c + 512]
                    return ptB[:, c:c + 512].bitcast(bf16)

                for m in range(4):
                    nc.tensor.matmul(
                        pv[32 * m:32 * m + 32, :],
                        lhsT=vt_sb[jc][:, (4 * tg + m) * 32:(4 * tg + m) * 32 + 32],
                        rhs=prhs(m),
                        start=False,
                        stop=(jc == 7),
                        tile_position=(0, 32 * m),
                    )
                for m in range(4):
                    nc.tensor.matmul(
                        dn[32 * m:32 * m + 32, :],
                        lhsT=ones_sb,
                        rhs=prhs(m),
                        start=False,
                        stop=(jc == 7),
                        tile_position=(0, 32 * m),
                    )
                if jc == 7:
                    deferred.extend(make_chain_a_parts(tg, ih, qi, pv, dn))
                    if tg == 1:
                        fine = (ih == 1)
                        deferred.extend(make_chain_b_parts(
                            ih, ((0, 256), (256, 256)), fine))

            return st_fn, pv_fn

        # ih-major quad order: both head groups of a column-half complete
        # before its projection, which then runs mid-window.
        quads = [(0, 0), (1, 0), (0, 1), (1, 1)]
        iter_hooks = {
            (0, 1): [lambda: vt_pair(0)],
            (0, 2): [lambda: vt_pair(2)],
            (0, 3): [lambda: qk_half(2, 1, use_act=True)],
            (0, 4): [lambda: vt_pair(4)],
            (0, 5): [lambda: qk_pair(1, use_act=True)],
            (0, 6): [lambda: qk_half(3, 0, use_act=True)],
            (0, 7): [lambda: vt_pair(6)],
            (1, 1): [lambda: qk_half(3, 1, use_act=True)],
            (1, 4): [lambda: qk_half(0, 1, use_act=True)],
        }
        fns = [make_quad(qi, tg, ih) for qi, (tg, ih) in enumerate(quads)]
        pv_slots = {}
        for q in range(4):
            for j in range(8):
                lag = 1 if (q == 3 and j >= 6) else 2
                pv_slots.setdefault(8 * q + j + lag, []).append((q, j))

        def emit_pvs(s):
            for pq, pj in pv_slots.get(s, []):
                fns[pq][1](pj)

        for s in range(32):
            q, r = divmod(s, 8)
            fns[q][0](r)
            if r in (2, 3, 5) and deferred:
                deferred.pop(0)()
            emit_pvs(s)
        for s in (32, 33):
            emit_pvs(s)
        while deferred:
            deferred.pop(0)()

    nc.compile()
    return nc


def _get_nc():
    if "nc" not in _CACHE:
        _CACHE["nc"] = _build_nc()
    return _CACHE["nc"]


def _pack_weights(w_qkv, b_qkv, w_proj, b_proj):
    w_qkv = np.asarray(w_qkv, dtype=np.float32)
    b_qkv = np.asarray(b_qkv, dtype=np.float32)
    w_proj = np.asarray(w_proj, dtype=np.float32)
    b_proj = np.asarray(b_proj, dtype=np.float32)

    wqkT = np.ascontiguousarray(w_qkv[:512].T)                  # (256, 512)
    bqkc = np.zeros((128, 4), dtype=np.float32)
    bqkc[:, 0:4] = b_qkv[:512].reshape(4, 128).T
    wvT = np.ascontiguousarray(w_qkv[512:768].T)                # (256, 256)
    wpT = np.ascontiguousarray(w_proj.T)                        # (256, 256)

    wext = np.zeros((128, XW_W - 2048), dtype=np.float32)
    wext[:, 0:512] = wqkT[0:128]
    wext[:, 512:1024] = wqkT[128:256]
    wext[:, 1024:1280] = wvT[0:128]
    wext[:, 1280:1536] = wvT[128:256]
    wext[:, 1536:1792] = wpT[0:128]
    wext[:, 1792:2048] = wpT[128:256]
    # 128x128 identity: residual preload lhsT for the projection PSUM group
    wext[:, 2048:2176] = np.eye(128, dtype=np.float32)
    # all-ones (128,32): denominator lhsT -- every row of a head band gets l
    wext[:, 2176:2208] = 1.0
    # cols 2208:2336 stay zero (zl: PSUM pre-clear lhsT)
    return wext, bqkc


def _bf16(a):
    import ml_dtypes

    return np.asarray(a).astype(ml_dtypes.bfloat16)


def _install_ntff_hook_module():
    """bass_utils wants antenv.axon_hooks for trace=True under axon; this
    image's antenv lacks it.  Inject an equivalent module into sys.modules."""
    if "antenv.axon_hooks" in sys.modules:
        return
    try:
        import antenv.axon_hooks  # noqa: F401

        return
    except ImportError:
        pass
    import contextlib
    import ctypes
    import types

    mod = types.ModuleType("antenv.axon_hooks")
    state = {"hook": None, "inited": False}

    def _default_hook():
        so_path = "/opt/axon/libaxon_pjrt.so"
        if not os.path.exists(so_path):
            return None
        lib = ctypes.CDLL(so_path)
        if not hasattr(lib, "axon_start_nrt_profile"):
            return None
        lib.axon_start_nrt_profile.argtypes = [
            ctypes.POINTER(ctypes.c_int64),
            ctypes.c_size_t,
        ]
        lib.axon_start_nrt_profile.restype = ctypes.c_int64
        lib.axon_stop_nrt_profile.argtypes = [ctypes.c_char_p]
        lib.axon_stop_nrt_profile.restype = ctypes.c_int64

        @contextlib.contextmanager
        def _hook(output_dir, device_ids):
            import jax

            jax.devices()
            if device_ids:
                ids = (ctypes.c_int64 * len(device_ids))(*device_ids)
                rc = lib.axon_start_nrt_profile(ids, len(device_ids))
            else:
                rc = lib.axon_start_nrt_profile(None, 0)
            if rc != 0:
                raise RuntimeError(f"axon_start_nrt_profile rc={rc}")
            try:
                yield
            finally:
                n = lib.axon_stop_nrt_profile(str(output_dir).encode())
                if n < 0:
                    raise RuntimeError(f"axon_stop_nrt_profile rc={n}")
                print(f"profile: {n} file(s) written to {output_dir}")

        return _hook

    def set_axon_ntff_profile_hook(hook):
        state["hook"] = hook
        state["inited"] = True

    def get_axon_ntff_profile_hook():
        if not state["inited"]:
            state["hook"] = _default_hook()
            state["inited"] = True
        return state["hook"]

    mod.set_axon_ntff_profile_hook = set_axon_ntff_profile_hook
    mod.get_axon_ntff_profile_hook = get_axon_ntff_profile_hook
    sys.modules["antenv.axon_hooks"] = mod


def _prepare_in_maps(x, w_qkv, b_qkv, w_proj, b_proj):
    import ml_dtypes

    x = np.asarray(x, dtype=np.float32)
    b, c, h, w = x.shape
    assert (b, c, h, w) == (B, C, 32, 32)

    wext, bqkc = _pack_weights(w_qkv, b_qkv, w_proj, b_proj)
    wext_bf = _bf16(wext)
    bqkc = np.ascontiguousarray(bqkc, dtype=np.float32)

    w_qkv = np.asarray(w_qkv, dtype=np.float32)
    w_proj = np.asarray(w_proj, dtype=np.float32)
    b_qkv = np.asarray(b_qkv, dtype=np.float32)
    b_proj = np.asarray(b_proj, dtype=np.float32)
    resid_bias = b_proj + w_proj @ b_qkv[512:768]   # (256,)

    in_maps = []
    for core in range(N_CORES):
        xm = np.ascontiguousarray(x[core].reshape(C, L))
        xb = _bf16(xm)
        # rb = bf16(x - bf16(x) + resid_bias): residual correction folded
        # with the projection/V biases, PSUM-accumulated via identity lhsT.
        rbm = _bf16(xm - xb.astype(np.float32) + resid_bias[:, None])
        xw = np.empty((128, XW_W), dtype=wext_bf.dtype)
        xw[:, 0:1024] = xb[0:128]
        xw[:, 1024:2048] = xb[128:256]
        xw[:, 2048:XW_W] = wext_bf
        rb = np.empty((128, 2048), dtype=wext_bf.dtype)
        rb[:, 0:1024] = rbm[0:128]
        rb[:, 1024:2048] = rbm[128:256]
        m = {"xw": xw, "rb": rb, "bqkc": bqkc}
        in_maps.append(m)
    return in_maps


def kernel(x, w_qkv, b_qkv, w_proj, b_proj, _trace=False, _trace_kwargs=None):
    if _trace:
        _install_ntff_hook_module()
    from concourse.bass_utils import run_bass_kernel_spmd

    in_maps = _prepare_in_maps(x, w_qkv, b_qkv, w_proj, b_proj)
    nc = _get_nc()

    res = run_bass_kernel_spmd(
        nc,
        in_maps,
        list(range(N_CORES)),
        trace=_trace,
        **(_trace_kwargs or {}),
    )
    out = np.stack([res.results[core]["out"] for core in range(N_CORES)])
    if _trace:
        _CACHE["last_result"] = res
    return out.reshape(B, C, 32, 32)
